# revision 32
# baseline (speedup 1.0000x reference)
"""Multi-head self-attention (RoPE, causal) Trainium2 Bass kernel.

Problem: B=4, S=2048, D=1024, H=16 heads, d_k=64, f32 in/out.

Sharding: head-parallel across 8 NeuronCores. Core c owns heads {2c, 2c+1}
and all batches/tokens. QKV projections are column-parallel, attention is
local per core, output projection is column-parallel after an AllGather of
the per-core attention outputs (each core computes 128 of 1024 y features).

Performance structure (v2):
  - software-pipelined phases: proj(b+1) and wo(b-1) instruction streams are
    interleaved into attention(b)'s strip loop so the tensor engine always
    has dense independent work (keeps PE HAM-warm at 2.4 GHz)
  - per-batch QR/KR/VA/ATT tiles ping-pong (bufs=2) so cross-batch phases
    carry no false dependencies
  - score matmuls are d_k=64-contraction: the two heads are issued
    back-to-back at PE row-tiles (0,0)/(64,0) and execute concurrently
  - P@V carries a ones column per head so the softmax denominator
    accumulates in PSUM row 64 for free; normalization is fused into the
    PSUM->ATT copy (per-strip reciprocal + selector-broadcast matmul)
  - AllGather per half batch, wo chunks interleaved one batch behind
  - input x chunks double-split DMAs with bufs=3 so projections never starve
"""

import numpy as np
import ml_dtypes
from contextlib import ExitStack

import concourse.bass as bass
import concourse.bacc as bacc
import concourse.tile as tile
from concourse import mybir
from concourse.bass_utils import run_bass_kernel_spmd
from concourse.masks import make_upper_triangular, make_identity

F32 = mybir.dt.float32
BF16 = mybir.dt.bfloat16
FP32R = mybir.dt.float32r

B, S, D, H = 4, 2048, 1024, 16
NC = 8
DK = 64
HPC = H // NC  # heads per core
THETA = 100000.0
QC = 512        # q chunk (tokens per score-strip column block)
KB = 128        # k block (tokens per score-strip partition block)
NDB = D // 128  # number of 128-wide contraction blocks
NQC = S // QC   # chunks per batch (4)
NKB = S // KB   # k blocks per batch (16)

ts = bass.ts
ds = bass.ds

EXP = mybir.ActivationFunctionType.Exp
SCALE = 1.0 / float(np.sqrt(DK))


def build_program(n_cores=NC, mm_dt=BF16):
    nc = bacc.Bacc("TRN2", target_bir_lowering=False, debug=False,
                   num_devices=n_cores)
    T = B * S

    xT_h = nc.declare_dram_parameter("xT", [D, T], mm_dt, isOutput=False)
    wqk_h = nc.declare_dram_parameter("wqkT", [128, 2, NDB, 128], mm_dt,
                                      isOutput=False)
    wv_h = nc.declare_dram_parameter("wvT", [128, NDB, 128], mm_dt,
                                     isOutput=False)
    wo_h = nc.declare_dram_parameter("woT", [128, NDB, 128], mm_dt,
                                     isOutput=False)
    ra_h = nc.declare_dram_parameter("ropeA", [128, S], mm_dt, isOutput=False)
    rb_h = nc.declare_dram_parameter("ropeB", [128, S], mm_dt, isOutput=False)
    sel_h = nc.declare_dram_parameter("sel2", [1, 2, 128], mm_dt,
                                     isOutput=False)
    yT_h = nc.declare_dram_parameter("yT", [128, T], F32, isOutput=True)

    with tile.TileContext(nc, num_cores=n_cores) as tc, ExitStack() as ctx:
        consts = ctx.enter_context(tc.tile_pool(name="consts", bufs=1))

        # wqk first: the first projection matmuls depend only on it + xc0
        wqks = consts.tile([128, 2, NDB, 128], mm_dt)
        for g in range(2):
            for hh in range(2):
                nc.sync.dma_start(wqks[:, g, ds(4 * hh, 4), :],
                                  wqk_h[:, g, ds(4 * hh, 4), :])
        ra_t = consts.tile([128, S], mm_dt)
        rb_t = consts.tile([128, S], mm_dt)
        wvs = consts.tile([128, NDB, 128], mm_dt)
        wos = consts.tile([128, NDB, 128], mm_dt)
        sel2 = consts.tile([1, 2, 128], mm_dt)
        triu = consts.tile([128, 128], mm_dt)
        make_upper_triangular(nc, triu[:], val=1.0, diag=True)
        ident = consts.tile([128, 128], mm_dt)
        make_identity(nc, ident[:])

        def load_consts():
            # issued after the first x chunk so its queues aren't blocked
            nc.sync.dma_start(wvs[:], wv_h[:, :, :])
            nc.sync.dma_start(ra_t[:], ra_h[:, :])
            nc.sync.dma_start(rb_t[:], rb_h[:, :])
            nc.sync.dma_start(wos[:], wo_h[:, :, :])
            nc.sync.dma_start(sel2[:], sel_h[:, :, :])

        xr = xT_h[:, :].rearrange("(i p) t -> p i t", p=128)

        with (tc.tile_pool(name="qrkr", bufs=2) as qrp,
              tc.tile_pool(name="vap", bufs=2) as vap,
              tc.tile_pool(name="attp", bufs=2) as attp,
              tc.tile_pool(name="xcp", bufs=4) as xcp,
              tc.tile_pool(name="xgp", bufs=2) as xgp,
              tc.tile_pool(name="xdp", bufs=2) as xdp,
              tc.tile_pool(name="ropet", bufs=2) as rpool,
              tc.tile_pool(name="spp", bufs=2, space="PSUM") as spp,
              tc.tile_pool(name="pvp", bufs=1, space="PSUM") as pvp,
              tc.tile_pool(name="mmp", bufs=2, space="PSUM") as mmp,
              tc.tile_pool(name="ptp", bufs=3) as ptp,
              tc.tile_pool(name="nrm", bufs=2) as nrm,
              tc.tile_pool(name="rtp", bufs=3) as rtp,
              tc.tile_pool(name="ysb", bufs=2) as ysbp,
              tc.tile_pool(name="dram", bufs=6, space="DRAM") as dpool):

            # per-batch SBUF state (ping-pong via pool bufs=2)
            qr_t = [None] * B
            kr_t = [None] * B
            va_t = [None] * B
            att_t = [None] * B
            attb = {}
            agb = {}

            def emit_proj_chunk(b, ci):
                """QKV projection + rope + V-transpose for 512 tokens."""
                if ci == 0:
                    qr_t[b] = qrp.tile([128, S], mm_dt, tag="qr", name="qr")
                    kr_t[b] = qrp.tile([128, S], mm_dt, tag="kr", name="kr")
                    va_t[b] = vap.tile([128, NKB, 130], mm_dt, tag="va",
                                       name="va")
                    att_t[b] = attp.tile([128, S], mm_dt, tag="att",
                                         name="att")
                    nc.vector.memset(va_t[b][:, :, 64], 1.0)
                    nc.vector.memset(va_t[b][:, :, 129], 1.0)
                QR, KR, VA = qr_t[b], kr_t[b], va_t[b]
                c = b * NQC + ci
                bsl = ds(ci * QC, QC)
                xc = xcp.tile([128, NDB, QC], mm_dt, tag="xc", name="xc")
                # split the 512KB load 4-way across DMA queues
                for sp_i in range(4):
                    nc.sync.dma_start(xc[:, ds(2 * sp_i, 2), :],
                                      xr[:, ds(2 * sp_i, 2), ts(c, QC)])
                if c == 0:
                    load_consts()

                xsb = []
                for g in range(2):
                    ps = mmp.tile([128, QC], F32, tag="mm", name="ps")
                    for i in range(NDB):
                        nc.tensor.matmul(ps[:], wqks[:, g, i, :], xc[:, i, :],
                                         start=(i == 0), stop=(i == NDB - 1))
                    xg = xgp.tile([128, QC], mm_dt, tag=f"xg{g}", name="xg")
                    nc.vector.tensor_copy(xg[:], ps[:])
                    xsb.append(xg)
                # row duplication via SBUF->SBUF DMA: slots [XA, XB, XC, XD]
                xd = xdp.tile([128, 4, QC], mm_dt, tag="xd", name="xd")
                for t_i, (src_t, base) in enumerate(
                        ((xsb[0], 0), (xsb[1], 0),
                         (xsb[0], 64), (xsb[1], 64))):
                    for blk in range(2):
                        for du in range(2):
                            nc.sync.dma_start(
                                xd[ds(64 * blk + 32 * du, 32), t_i, :],
                                src_t[ds(base + 32 * blk, 32), :])
                for xi, OUT in ((0, QR), (2, KR)):
                    t1 = rpool.tile([128, QC], mm_dt, tag="t1", name="t1")
                    t2 = rpool.tile([128, QC], mm_dt, tag="t2", name="t2")
                    nc.vector.tensor_mul(t1[:], xd[:, xi, :], ra_t[:, bsl])
                    nc.vector.tensor_mul(t2[:], xd[:, xi + 1, :],
                                         rb_t[:, bsl])
                    nc.vector.tensor_add(OUT[:, bsl], t1[:], t2[:])

                # V: project transposed, then xbar-DMA-transpose into VA
                psv = mmp.tile([128, QC], F32, tag="mm", name="psv")
                for i in range(NDB):
                    nc.tensor.matmul(psv[:], wvs[:, i, :], xc[:, i, :],
                                     start=(i == 0), stop=(i == NDB - 1))
                vtc = xgp.tile([128, QC], mm_dt, tag="vtc", name="vtc")
                nc.vector.tensor_copy(vtc[:], psv[:])
                for sb in range(QC // 128):
                    tb = ci * (QC // 128) + sb
                    tp = mmp.tile([128, 128], mm_dt, tag="mm", name="tp")
                    nc.tensor.transpose(tp[:], vtc[:, ts(sb, 128)], ident[:])
                    # one strided copy: cols {0:64} and {65:129} of VA
                    dst = VA[:, tb, 0:130].rearrange("p (a c) -> p a c", a=2)
                    src = tp[:, :].rearrange("p (a c) -> p a c", a=2)
                    nc.vector.tensor_copy(dst[:, :, 0:64], src[:, :, :])

            def emit_attn_strip(b, qi):
                """Scores+softmax+PV for q tokens [qi*QC, qi*QC+QC), both
                heads; normalization fused into the PSUM->ATT copy."""
                QR, KR, VA, ATT = qr_t[b], kr_t[b], va_t[b], att_t[b]
                nk = (qi + 1) * (QC // KB)
                pvs = [pvp.tile([65, QC], F32, tag=f"pv{h}", name="pv")
                       for h in range(2)]
                for kb in range(nk):
                    dj = kb - qi * (QC // KB)
                    qv = 128 * dj if dj > 0 else 0
                    ksl = ds(kb * KB, KB)
                    qsl = ds(qi * QC + qv, QC - qv)
                    sp = spp.tile([128, 2, QC], F32, tag="sp", name="sp")
                    # two heads back-to-back at PE row tiles (0,0)/(64,0)
                    for h in range(2):
                        hr = ds(64 * h, 64)
                        nc.tensor.matmul(sp[:, h, qv:QC],
                                         KR[hr, ksl], QR[hr, qsl],
                                         start=True, stop=True)
                    pt = ptp.tile([128, 2, QC], mm_dt, tag="pt", name="pt")
                    nc.scalar.activation(pt[:, :, qv:QC], sp[:, :, qv:QC],
                                         EXP, scale=SCALE)
                    if dj >= 0:  # diagonal block: causal mask
                        dsl = ds(128 * dj, 128)
                        for h in range(2):
                            nc.vector.tensor_mul(pt[:, h, dsl],
                                                 pt[:, h, dsl], triu[:])
                    for h in range(2):
                        nc.tensor.matmul(pvs[h][:, qv:QC],
                                         VA[:, kb, ds(65 * h, 65)],
                                         pt[:, h, qv:QC],
                                         start=(kb == 0),
                                         stop=(kb == nk - 1))
                # per-strip normalize: denominators sit in pv row 64;
                # bp built from two rank-1 accumulating matmuls (no DMA)
                qsl = ds(qi * QC, QC)
                bp = mmp.tile([128, QC], F32, tag="mm", name="bp")
                for h in range(2):
                    deng = nrm.tile([1, QC], F32, tag=f"deng{h}", name="deng")
                    nc.scalar.copy(deng[:], pvs[h][ds(64, 1), :])
                    nc.vector.tensor_copy(ATT[ds(64 * h, 64), qsl],
                                          pvs[h][ds(0, 64), :])
                    rcp = nrm.tile([1, QC], F32, tag=f"rcp{h}", name="rcp")
                    nc.vector.reciprocal_approx_fast(rcp[:], deng[:])
                    rcb = nrm.tile([1, QC], mm_dt, tag=f"rcb{h}", name="rcb")
                    nc.vector.tensor_copy(rcb[:], rcp[:])
                    nc.tensor.matmul(bp[:], sel2[:, h, :], rcb[:],
                                     start=(h == 0), stop=(h == 1))
                nc.vector.tensor_mul(ATT[:, qsl], ATT[:, qsl], bp[:])
                # stage this strip into the half-batch AllGather input
                half = qi // 2
                if qi % 2 == 0:
                    attb[(b, half)] = dpool.tile([128, S // 2], mm_dt,
                                                 tag="attb",
                                                 name=f"attb{b}_{half}")
                for sp_i in range(2):
                    tsl = ds((qi % 2) * QC + sp_i * (QC // 2), QC // 2)
                    ssl = ds(qi * QC + sp_i * (QC // 2), QC // 2)
                    nc.sync.dma_start(attb[(b, half)][:, tsl], ATT[:, ssl])

            def emit_ag(b, half):
                """AllGather one half-batch of attention output."""
                ag = dpool.tile([128 * n_cores, S // 2], mm_dt,
                                addr_space="Shared", tag="agb",
                                name=f"agb{b}_{half}")
                agb[(b, half)] = ag
                nc.gpsimd.collective_compute(
                    "AllGather", mybir.AluOpType.bypass,
                    replica_groups=[list(range(n_cores))],
                    ins=[attb[(b, half)][:, :].opt()], outs=[ag[:, :].opt()])

            def emit_wo_chunk(b, half, cl):
                """Output projection for 512 tokens of an AllGathered half."""
                agr = agb[(b, half)][:, :].rearrange("(i p) t -> p i t",
                                                     p=128)
                rt = rtp.tile([128, NDB, QC], mm_dt, tag="rt", name="rt")
                for sp_i in range(4):
                    nc.sync.dma_start(rt[:, ds(2 * sp_i, 2), :],
                                      agr[:, ds(2 * sp_i, 2), ts(cl, QC)])
                yp = mmp.tile([128, QC], F32, tag="mm", name="yp")
                for i in range(NDB):
                    nc.tensor.matmul(yp[:], wos[:, i, :], rt[:, i, :],
                                     start=(i == 0), stop=(i == NDB - 1))
                ysb_t = ysbp.tile([128, QC], F32, tag="ys", name="ysb_t")
                nc.vector.tensor_copy(ysb_t[:], yp[:])
                tok0 = b * S + half * (S // 2) + cl * QC
                nc.sync.dma_start(yT_h[:, ds(tok0, QC // 2)],
                                  ysb_t[:, 0:QC // 2])
                nc.sync.dma_start(yT_h[:, ds(tok0 + QC // 2, QC // 2)],
                                  ysb_t[:, QC // 2:QC])

            # ---- pipelined schedule -------------------------------------
            # proj runs ~3 strip-slots ahead of its consumer strip; wo runs
            # two batch-windows behind so 8 wo chunks remain to fill the
            # final AllGather window.
            for ci in range(NQC):
                emit_proj_chunk(0, ci)
            emit_proj_chunk(1, 0)
            # (batch, qi) -> proj chunk to emit after that strip
            proj_slots = {}
            slots = [(b, qi) for b in range(B) for qi in range(NQC)]
            chunks = ([(1, ci) for ci in range(1, NQC)]
                      + [(2, ci) for ci in range(NQC)]
                      + [(3, ci) for ci in range(NQC)])
            # chunk (3,3) lands in A(3) slot 0; others pack earlier slots
            for slot, chk in zip(slots, chunks[:-1]):
                proj_slots[slot] = chk
            proj_slots[(3, 0)] = (3, 3)
            for b in range(B):
                for qi in range(NQC):
                    emit_attn_strip(b, qi)
                    if qi == 1:
                        emit_ag(b, 0)
                    if qi == 3:
                        emit_ag(b, 1)
                    if (b, qi) in proj_slots:
                        pb, pci = proj_slots[(b, qi)]
                        emit_proj_chunk(pb, pci)
                    if b >= 2:
                        emit_wo_chunk(b - 2, qi // 2, qi % 2)
            # tail: wo of batches 2 and 3 fills the last AllGather window
            for wb in (2, 3):
                for half in range(2):
                    for cl in range(2):
                        emit_wo_chunk(wb, half, cl)

    nc.compile()
    return nc


def prep_inputs(inputs, mm_dt=BF16, n_cores=NC):
    """Host-side sharding: build the per-core input maps."""
    mm_np = ml_dtypes.bfloat16 if mm_dt == BF16 else np.float32
    x = np.asarray(inputs["in_features"], dtype=np.float32)
    pos = np.asarray(inputs["token_positions"]).astype(np.float32)
    wq = np.asarray(inputs["w_q"], dtype=np.float32)
    wk = np.asarray(inputs["w_k"], dtype=np.float32)
    wv = np.asarray(inputs["w_v"], dtype=np.float32)
    wo = np.asarray(inputs["w_o"], dtype=np.float32)

    T = B * S
    xT = np.ascontiguousarray(x.reshape(T, D).T).astype(mm_np)

    # rope tables: QR = XA*A + XB*B ; A rows per 32-block: [cos, sin]*2 ;
    # B rows: [-sin, cos]*2
    inv = np.float32(THETA) ** (-np.arange(0, DK, 2, dtype=np.float32)
                                / np.float32(DK))
    ang = pos[:, None].astype(np.float32) * inv[None, :].astype(np.float32)
    cosT = np.cos(ang.astype(np.float32)).T  # [32, S]
    sinT = np.sin(ang.astype(np.float32)).T
    ropeA = np.ascontiguousarray(
        np.concatenate([cosT, sinT, cosT, sinT], axis=0)).astype(mm_np)
    ropeB = np.ascontiguousarray(
        np.concatenate([-sinT, cosT, -sinT, cosT], axis=0)).astype(mm_np)

    # normalize broadcast selector: out row m takes denom row m // 64
    sel2 = np.zeros((1, 2, 128), dtype=np.float32)
    sel2[0, 0, 0:64] = 1.0
    sel2[0, 1, 64:128] = 1.0

    ev = np.arange(0, DK, 2)
    od = ev + 1

    def lhsT_stack(W):
        # W [128 out, D] -> [128, NDB, 128]; [:, i, :] = W[:, 128i:+128].T
        Wt = np.ascontiguousarray(W.T).astype(mm_np)  # [D, 128]
        return np.ascontiguousarray(
            Wt.reshape(NDB, 128, 128).transpose(1, 0, 2))

    in_maps = []
    for c in range(n_cores):
        h0, h1 = HPC * c, HPC * c + 1
        W1 = np.concatenate([wq[DK * h0 + ev], wq[DK * h1 + ev],
                             wk[DK * h0 + ev], wk[DK * h1 + ev]], axis=0)
        W2 = np.concatenate([wq[DK * h0 + od], wq[DK * h1 + od],
                             wk[DK * h0 + od], wk[DK * h1 + od]], axis=0)
        wqkT = np.ascontiguousarray(np.stack(
            [lhsT_stack(Wg) for Wg in (W1, W2)], axis=1))
        WV = wv[128 * c: 128 * (c + 1)]
        WO = wo[128 * c: 128 * (c + 1)]
        in_maps.append({
            "xT": xT,
            "wqkT": wqkT,
            "wvT": lhsT_stack(WV),
            "woT": lhsT_stack(WO),
            "ropeA": ropeA,
            "ropeB": ropeB,
            "sel2": sel2.astype(ml_dtypes.bfloat16),
        })
    return in_maps


def assemble_output(results, n_cores=NC):
    yT = np.concatenate([np.asarray(r["yT"], dtype=np.float32)
                         for r in results], axis=0)  # [1024, T]
    return np.ascontiguousarray(yT.T).reshape(B, S, D).astype(np.float32)


_PROGRAM_CACHE = {}


def kernel(**inputs) -> np.ndarray:
    key = ("v2", S, "bf16")
    if key not in _PROGRAM_CACHE:
        _PROGRAM_CACHE[key] = build_program(n_cores=NC, mm_dt=BF16)
    nc = _PROGRAM_CACHE[key]
    in_maps = prep_inputs(inputs, mm_dt=BF16, n_cores=NC)
    res = run_bass_kernel_spmd(nc, in_maps, list(range(NC)))
    return assemble_output(res.results)


# revision 35
# speedup vs baseline: 1.0335x; 1.0335x over previous
"""Multi-head self-attention (RoPE, causal) Trainium2 Bass kernel.

Problem: B=4, S=2048, D=1024, H=16 heads, d_k=64, f32 in/out.

Sharding: head-parallel across 8 NeuronCores. Core c owns heads {2c, 2c+1}
and all batches/tokens. QKV projections are column-parallel, attention is
local per core, output projection is column-parallel after an AllGather of
the per-core attention outputs (each core computes 128 of 1024 y features).

Performance structure (v2):
  - software-pipelined phases: proj(b+1) and wo(b-1) instruction streams are
    interleaved into attention(b)'s strip loop so the tensor engine always
    has dense independent work (keeps PE HAM-warm at 2.4 GHz)
  - per-batch QR/KR/VA/ATT tiles ping-pong (bufs=2) so cross-batch phases
    carry no false dependencies
  - score matmuls are d_k=64-contraction: the two heads are issued
    back-to-back at PE row-tiles (0,0)/(64,0) and execute concurrently
  - P@V carries a ones column per head so the softmax denominator
    accumulates in PSUM row 64 for free; normalization is fused into the
    PSUM->ATT copy (per-strip reciprocal + selector-broadcast matmul)
  - AllGather per half batch, wo chunks interleaved one batch behind
  - input x chunks double-split DMAs with bufs=3 so projections never starve
"""

import numpy as np
import ml_dtypes
from contextlib import ExitStack

import concourse.bass as bass
import concourse.bacc as bacc
import concourse.tile as tile
from concourse import mybir
from concourse.bass_utils import run_bass_kernel_spmd
from concourse.masks import make_upper_triangular, make_identity

F32 = mybir.dt.float32
BF16 = mybir.dt.bfloat16
FP32R = mybir.dt.float32r

B, S, D, H = 4, 2048, 1024, 16
NC = 8
DK = 64
HPC = H // NC  # heads per core
THETA = 100000.0
QC = 512        # q chunk (tokens per score-strip column block)
KB = 128        # k block (tokens per score-strip partition block)
NDB = D // 128  # number of 128-wide contraction blocks
NQC = S // QC   # chunks per batch (4)
NKB = S // KB   # k blocks per batch (16)

ts = bass.ts
ds = bass.ds

EXP = mybir.ActivationFunctionType.Exp
SCALE = 1.0 / float(np.sqrt(DK))


def build_program(n_cores=NC, mm_dt=BF16):
    nc = bacc.Bacc("TRN2", target_bir_lowering=False, debug=False,
                   num_devices=n_cores)
    T = B * S

    xT_h = nc.declare_dram_parameter("xT", [D, T], mm_dt, isOutput=False)
    wqk_h = nc.declare_dram_parameter("wqkT", [128, 2, NDB, 128], mm_dt,
                                      isOutput=False)
    wv_h = nc.declare_dram_parameter("wvT", [128, NDB, 128], mm_dt,
                                     isOutput=False)
    wo_h = nc.declare_dram_parameter("woT", [128, NDB, 128], mm_dt,
                                     isOutput=False)
    ra_h = nc.declare_dram_parameter("ropeA", [128, S], mm_dt, isOutput=False)
    rb_h = nc.declare_dram_parameter("ropeB", [128, S], mm_dt, isOutput=False)
    sel_h = nc.declare_dram_parameter("sel2", [1, 2, 128], mm_dt,
                                     isOutput=False)
    yT_h = nc.declare_dram_parameter("yT", [128, T], F32, isOutput=True)

    with tile.TileContext(nc, num_cores=n_cores) as tc, ExitStack() as ctx:
        consts = ctx.enter_context(tc.tile_pool(name="consts", bufs=1))

        # wqk first: the first projection matmuls depend only on it + xc0
        wqks = consts.tile([128, 2, NDB, 128], mm_dt)
        for g in range(2):
            for hh in range(2):
                nc.sync.dma_start(wqks[:, g, ds(4 * hh, 4), :],
                                  wqk_h[:, g, ds(4 * hh, 4), :])
        ra_t = consts.tile([128, S], mm_dt)
        nc.sync.dma_start(ra_t[:], ra_h[:, :])
        rb_t = consts.tile([128, S], mm_dt)
        nc.sync.dma_start(rb_t[:], rb_h[:, :])
        wvs = consts.tile([128, NDB, 128], mm_dt)
        nc.sync.dma_start(wvs[:], wv_h[:, :, :])
        wos = consts.tile([128, NDB, 128], mm_dt)
        nc.sync.dma_start(wos[:], wo_h[:, :, :])
        sel2 = consts.tile([1, 2, 128], mm_dt)
        nc.sync.dma_start(sel2[:], sel_h[:, :, :])
        triu = consts.tile([128, 128], mm_dt)
        make_upper_triangular(nc, triu[:], val=1.0, diag=True)
        ident = consts.tile([128, 128], mm_dt)
        make_identity(nc, ident[:])

        xr = xT_h[:, :].rearrange("(i p) t -> p i t", p=128)

        with (tc.tile_pool(name="qrkr", bufs=2) as qrp,
              tc.tile_pool(name="vap", bufs=2) as vap,
              tc.tile_pool(name="attp", bufs=2) as attp,
              tc.tile_pool(name="xcp", bufs=4) as xcp,
              tc.tile_pool(name="xgp", bufs=2) as xgp,
              tc.tile_pool(name="xdp", bufs=2) as xdp,
              tc.tile_pool(name="ropet", bufs=2) as rpool,
              tc.tile_pool(name="spp", bufs=2, space="PSUM") as spp,
              tc.tile_pool(name="pvp", bufs=1, space="PSUM") as pvp,
              tc.tile_pool(name="mmp", bufs=2, space="PSUM") as mmp,
              tc.tile_pool(name="ptp", bufs=3) as ptp,
              tc.tile_pool(name="nrm", bufs=2) as nrm,
              tc.tile_pool(name="rtp", bufs=3) as rtp,
              tc.tile_pool(name="ysb", bufs=2) as ysbp,
              tc.tile_pool(name="dram", bufs=6, space="DRAM") as dpool):

            # per-batch SBUF state (ping-pong via pool bufs=2)
            qr_t = [None] * B
            kr_t = [None] * B
            va_t = [None] * B
            att_t = [None] * B
            attb = {}
            agb = {}

            def emit_proj_chunk(b, ci):
                """QKV projection + rope + V-transpose for 512 tokens."""
                if ci == 0:
                    qr_t[b] = qrp.tile([128, S], mm_dt, tag="qr", name="qr")
                    kr_t[b] = qrp.tile([128, S], mm_dt, tag="kr", name="kr")
                    va_t[b] = vap.tile([128, NKB, 130], mm_dt, tag="va",
                                       name="va")
                    att_t[b] = attp.tile([128, S], mm_dt, tag="att",
                                         name="att")
                    nc.vector.memset(va_t[b][:, :, 64], 1.0)
                    nc.vector.memset(va_t[b][:, :, 129], 1.0)
                QR, KR, VA = qr_t[b], kr_t[b], va_t[b]
                c = b * NQC + ci
                bsl = ds(ci * QC, QC)
                xc = xcp.tile([128, NDB, QC], mm_dt, tag="xc", name="xc")
                # split the 512KB load across two DMA queues
                nc.sync.dma_start(xc[:, 0:4, :], xr[:, 0:4, ts(c, QC)])
                nc.sync.dma_start(xc[:, 4:8, :], xr[:, 4:8, ts(c, QC)])

                xsb = []
                for g in range(2):
                    ps = mmp.tile([128, QC], F32, tag="mm", name="ps")
                    for i in range(NDB):
                        nc.tensor.matmul(ps[:], wqks[:, g, i, :], xc[:, i, :],
                                         start=(i == 0), stop=(i == NDB - 1))
                    xg = xgp.tile([128, QC], mm_dt, tag=f"xg{g}", name="xg")
                    nc.vector.tensor_copy(xg[:], ps[:])
                    xsb.append(xg)
                # row duplication via SBUF->SBUF DMA: slots [XA, XB, XC, XD]
                xd = xdp.tile([128, 4, QC], mm_dt, tag="xd", name="xd")
                for t_i, (src_t, base) in enumerate(
                        ((xsb[0], 0), (xsb[1], 0),
                         (xsb[0], 64), (xsb[1], 64))):
                    for blk in range(2):
                        for du in range(2):
                            nc.sync.dma_start(
                                xd[ds(64 * blk + 32 * du, 32), t_i, :],
                                src_t[ds(base + 32 * blk, 32), :])
                for xi, OUT in ((0, QR), (2, KR)):
                    t1 = rpool.tile([128, QC], mm_dt, tag="t1", name="t1")
                    t2 = rpool.tile([128, QC], mm_dt, tag="t2", name="t2")
                    nc.vector.tensor_mul(t1[:], xd[:, xi, :], ra_t[:, bsl])
                    nc.vector.tensor_mul(t2[:], xd[:, xi + 1, :],
                                         rb_t[:, bsl])
                    nc.vector.tensor_add(OUT[:, bsl], t1[:], t2[:])

                # V: project transposed, then xbar-DMA-transpose into VA
                psv = mmp.tile([128, QC], F32, tag="mm", name="psv")
                for i in range(NDB):
                    nc.tensor.matmul(psv[:], wvs[:, i, :], xc[:, i, :],
                                     start=(i == 0), stop=(i == NDB - 1))
                vtc = xgp.tile([128, QC], mm_dt, tag="vtc", name="vtc")
                nc.vector.tensor_copy(vtc[:], psv[:])
                for sb in range(QC // 128):
                    tb = ci * (QC // 128) + sb
                    tp = mmp.tile([128, 128], mm_dt, tag="mm", name="tp")
                    nc.tensor.transpose(tp[:], vtc[:, ts(sb, 128)], ident[:])
                    # one strided copy: cols {0:64} and {65:129} of VA
                    dst = VA[:, tb, 0:130].rearrange("p (a c) -> p a c", a=2)
                    src = tp[:, :].rearrange("p (a c) -> p a c", a=2)
                    nc.vector.tensor_copy(dst[:, :, 0:64], src[:, :, :])

            def emit_attn_strip(b, qi):
                """Scores+softmax+PV for q tokens [qi*QC, qi*QC+QC), both
                heads; normalization fused into the PSUM->ATT copy."""
                QR, KR, VA, ATT = qr_t[b], kr_t[b], va_t[b], att_t[b]
                nk = (qi + 1) * (QC // KB)
                pvs = [pvp.tile([65, QC], F32, tag=f"pv{h}", name="pv")
                       for h in range(2)]
                for kb in range(nk):
                    dj = kb - qi * (QC // KB)
                    qv = 128 * dj if dj > 0 else 0
                    ksl = ds(kb * KB, KB)
                    qsl = ds(qi * QC + qv, QC - qv)
                    sp = spp.tile([128, 2, QC], F32, tag="sp", name="sp")
                    # two heads back-to-back at PE row tiles (0,0)/(64,0)
                    for h in range(2):
                        hr = ds(64 * h, 64)
                        nc.tensor.matmul(sp[:, h, qv:QC],
                                         KR[hr, ksl], QR[hr, qsl],
                                         start=True, stop=True)
                    pt = ptp.tile([128, 2, QC], mm_dt, tag="pt", name="pt")
                    nc.scalar.activation(pt[:, :, qv:QC], sp[:, :, qv:QC],
                                         EXP, scale=SCALE)
                    if dj >= 0:  # diagonal block: causal mask
                        dsl = ds(128 * dj, 128)
                        for h in range(2):
                            nc.vector.tensor_mul(pt[:, h, dsl],
                                                 pt[:, h, dsl], triu[:])
                    for h in range(2):
                        nc.tensor.matmul(pvs[h][:, qv:QC],
                                         VA[:, kb, ds(65 * h, 65)],
                                         pt[:, h, qv:QC],
                                         start=(kb == 0),
                                         stop=(kb == nk - 1))
                # per-strip normalize: denominators sit in pv row 64;
                # bp built from two rank-1 accumulating matmuls (no DMA)
                qsl = ds(qi * QC, QC)
                bp = mmp.tile([128, QC], F32, tag="mm", name="bp")
                for h in range(2):
                    deng = nrm.tile([1, QC], F32, tag=f"deng{h}", name="deng")
                    nc.scalar.copy(deng[:], pvs[h][ds(64, 1), :])
                    nc.vector.tensor_copy(ATT[ds(64 * h, 64), qsl],
                                          pvs[h][ds(0, 64), :])
                    rcp = nrm.tile([1, QC], F32, tag=f"rcp{h}", name="rcp")
                    nc.vector.reciprocal_approx_fast(rcp[:], deng[:])
                    rcb = nrm.tile([1, QC], mm_dt, tag=f"rcb{h}", name="rcb")
                    nc.vector.tensor_copy(rcb[:], rcp[:])
                    nc.tensor.matmul(bp[:], sel2[:, h, :], rcb[:],
                                     start=(h == 0), stop=(h == 1))
                nc.vector.tensor_mul(ATT[:, qsl], ATT[:, qsl], bp[:])
                # stage this strip into the half-batch AllGather input
                half = qi // 2
                if qi % 2 == 0:
                    attb[(b, half)] = dpool.tile([128, S // 2], mm_dt,
                                                 tag="attb",
                                                 name=f"attb{b}_{half}")
                for sp_i in range(2):
                    tsl = ds((qi % 2) * QC + sp_i * (QC // 2), QC // 2)
                    ssl = ds(qi * QC + sp_i * (QC // 2), QC // 2)
                    nc.sync.dma_start(attb[(b, half)][:, tsl], ATT[:, ssl])

            def emit_ag(b, half):
                """AllGather one half-batch of attention output."""
                ag = dpool.tile([128 * n_cores, S // 2], mm_dt,
                                addr_space="Shared", tag="agb",
                                name=f"agb{b}_{half}")
                agb[(b, half)] = ag
                nc.gpsimd.collective_compute(
                    "AllGather", mybir.AluOpType.bypass,
                    replica_groups=[list(range(n_cores))],
                    ins=[attb[(b, half)][:, :].opt()], outs=[ag[:, :].opt()])

            def emit_wo_chunk(b, half, cl):
                """Output projection for 512 tokens of an AllGathered half."""
                agr = agb[(b, half)][:, :].rearrange("(i p) t -> p i t",
                                                     p=128)
                rt = rtp.tile([128, NDB, QC], mm_dt, tag="rt", name="rt")
                nc.sync.dma_start(rt[:, 0:4, :], agr[:, 0:4, ts(cl, QC)])
                nc.sync.dma_start(rt[:, 4:8, :], agr[:, 4:8, ts(cl, QC)])
                yp = mmp.tile([128, QC], F32, tag="mm", name="yp")
                for i in range(NDB):
                    nc.tensor.matmul(yp[:], wos[:, i, :], rt[:, i, :],
                                     start=(i == 0), stop=(i == NDB - 1))
                ysb_t = ysbp.tile([128, QC], F32, tag="ys", name="ysb_t")
                nc.vector.tensor_copy(ysb_t[:], yp[:])
                tok0 = b * S + half * (S // 2) + cl * QC
                nc.sync.dma_start(yT_h[:, ds(tok0, QC // 2)],
                                  ysb_t[:, 0:QC // 2])
                nc.sync.dma_start(yT_h[:, ds(tok0 + QC // 2, QC // 2)],
                                  ysb_t[:, QC // 2:QC])

            # ---- pipelined schedule -------------------------------------
            # proj runs ~3 strip-slots ahead of its consumer strip; wo runs
            # two batch-windows behind so 8 wo chunks remain to fill the
            # final AllGather window.
            for ci in range(NQC):
                emit_proj_chunk(0, ci)
            emit_proj_chunk(1, 0)
            # (batch, qi) -> proj chunk to emit after that strip
            proj_slots = {}
            slots = [(b, qi) for b in range(B) for qi in range(NQC)]
            chunks = ([(1, ci) for ci in range(1, NQC)]
                      + [(2, ci) for ci in range(NQC)]
                      + [(3, ci) for ci in range(NQC)])
            # chunk (3,3) lands in A(3) slot 0; others pack earlier slots
            for slot, chk in zip(slots, chunks[:-1]):
                proj_slots[slot] = chk
            proj_slots[(3, 0)] = (3, 3)
            for b in range(B):
                for qi in range(NQC):
                    emit_attn_strip(b, qi)
                    if qi == 1:
                        emit_ag(b, 0)
                    if qi == 3:
                        emit_ag(b, 1)
                    if (b, qi) in proj_slots:
                        pb, pci = proj_slots[(b, qi)]
                        emit_proj_chunk(pb, pci)
                    if b >= 2:
                        emit_wo_chunk(b - 2, qi // 2, qi % 2)
            # tail: wo of batches 2 and 3 fills the last AllGather window
            for wb in (2, 3):
                for half in range(2):
                    for cl in range(2):
                        emit_wo_chunk(wb, half, cl)

    nc.compile()
    return nc


def prep_inputs(inputs, mm_dt=BF16, n_cores=NC):
    """Host-side sharding: build the per-core input maps."""
    mm_np = ml_dtypes.bfloat16 if mm_dt == BF16 else np.float32
    x = np.asarray(inputs["in_features"], dtype=np.float32)
    pos = np.asarray(inputs["token_positions"]).astype(np.float32)
    wq = np.asarray(inputs["w_q"], dtype=np.float32)
    wk = np.asarray(inputs["w_k"], dtype=np.float32)
    wv = np.asarray(inputs["w_v"], dtype=np.float32)
    wo = np.asarray(inputs["w_o"], dtype=np.float32)

    T = B * S
    xT = np.ascontiguousarray(x.reshape(T, D).T).astype(mm_np)

    # rope tables: QR = XA*A + XB*B ; A rows per 32-block: [cos, sin]*2 ;
    # B rows: [-sin, cos]*2
    inv = np.float32(THETA) ** (-np.arange(0, DK, 2, dtype=np.float32)
                                / np.float32(DK))
    ang = pos[:, None].astype(np.float32) * inv[None, :].astype(np.float32)
    cosT = np.cos(ang.astype(np.float32)).T  # [32, S]
    sinT = np.sin(ang.astype(np.float32)).T
    ropeA = np.ascontiguousarray(
        np.concatenate([cosT, sinT, cosT, sinT], axis=0)).astype(mm_np)
    ropeB = np.ascontiguousarray(
        np.concatenate([-sinT, cosT, -sinT, cosT], axis=0)).astype(mm_np)

    # normalize broadcast selector: out row m takes denom row m // 64
    sel2 = np.zeros((1, 2, 128), dtype=np.float32)
    sel2[0, 0, 0:64] = 1.0
    sel2[0, 1, 64:128] = 1.0

    ev = np.arange(0, DK, 2)
    od = ev + 1

    def lhsT_stack(W):
        # W [128 out, D] -> [128, NDB, 128]; [:, i, :] = W[:, 128i:+128].T
        Wt = np.ascontiguousarray(W.T).astype(mm_np)  # [D, 128]
        return np.ascontiguousarray(
            Wt.reshape(NDB, 128, 128).transpose(1, 0, 2))

    in_maps = []
    for c in range(n_cores):
        h0, h1 = HPC * c, HPC * c + 1
        W1 = np.concatenate([wq[DK * h0 + ev], wq[DK * h1 + ev],
                             wk[DK * h0 + ev], wk[DK * h1 + ev]], axis=0)
        W2 = np.concatenate([wq[DK * h0 + od], wq[DK * h1 + od],
                             wk[DK * h0 + od], wk[DK * h1 + od]], axis=0)
        wqkT = np.ascontiguousarray(np.stack(
            [lhsT_stack(Wg) for Wg in (W1, W2)], axis=1))
        WV = wv[128 * c: 128 * (c + 1)]
        WO = wo[128 * c: 128 * (c + 1)]
        in_maps.append({
            "xT": xT,
            "wqkT": wqkT,
            "wvT": lhsT_stack(WV),
            "woT": lhsT_stack(WO),
            "ropeA": ropeA,
            "ropeB": ropeB,
            "sel2": sel2.astype(ml_dtypes.bfloat16),
        })
    return in_maps


def assemble_output(results, n_cores=NC):
    yT = np.concatenate([np.asarray(r["yT"], dtype=np.float32)
                         for r in results], axis=0)  # [1024, T]
    return np.ascontiguousarray(yT.T).reshape(B, S, D).astype(np.float32)


_PROGRAM_CACHE = {}


def kernel(**inputs) -> np.ndarray:
    key = ("v2", S, "bf16")
    if key not in _PROGRAM_CACHE:
        _PROGRAM_CACHE[key] = build_program(n_cores=NC, mm_dt=BF16)
    nc = _PROGRAM_CACHE[key]
    in_maps = prep_inputs(inputs, mm_dt=BF16, n_cores=NC)
    res = run_bass_kernel_spmd(nc, in_maps, list(range(NC)))
    return assemble_output(res.results)
